# revision 46
# baseline (speedup 1.0000x reference)
"""Trainium2 Bass kernel for nn_Attention_16174846837077 (sparse_attention).

Math restructuring vs the reference:
  - The reference runs MHA 4 times (1 large + 3 gumbel-gated small branches with
    heads >= H//alpha zeroed).  A zeroed head has q=k=v=0 => its MHA output is 0,
    so branch i's MHA equals the full MHA with channels >= C//alpha_i zeroed.
  - Therefore small_sum = sum_i gw_i * ((mha * mask_i) @ W_proj.T + b_proj) * colmask_i
    collapses into ONE matmul with a block-rescaled weight matrix:
        Wtilde[j,c] = W_proj[j,c] * (gw0 + gw1*[j<384][c<384] + gw2*[j<256][c<256])
        btilde[j]   = b_proj[j]   * (gw0 + gw1*[j<384]        + gw2*[j<256])
  - Every batch then needs: QKV proj, full 12-head attention, one output proj.
    Data-parallel: batch b -> core b.  Cores 0-3 use (W_proj, b_proj), cores 4-7
    use (Wtilde, btilde).  Same SPMD graph, different weight data.  No collectives.

Device layout choices (all matmuls contract over the partition axis):
  - x is fed pre-transposed as xT (C, N) bf16.
  - q,k computed in transposed layout (d, n) so scoresT = kT.T @ qT directly.
  - v computed in natural layout (m, head, 96): 64 data cols + a ones column
    at 64 + zero padding, so attnV gives (96, n): rows 0-63 unnormalized out^T,
    row 64 = softmax denominators (exp-sums), rows 65-95 zeros (these make the
    32-row stream-transpose slice fully initialized).  Softmax skips
    max-subtraction (|score*scale| < ~4 for this data distribution).
  - normalization: the sums row is spread over 32 partitions with a DVE 32x32
    stream-transpose (reciprocal cost scales with free-size-per-lane), DVE
    reciprocal on the strided view, transpose back, gpsimd partition_broadcast
    to 64 rows, one elementwise multiply -> mhaT (c, n).
  - proj consumes mhaT directly as the stationary operand; the bias is a
    DMA-partition-broadcast (128, C) tile folded into the PSUM-drain add.
  - scheduling: head pair 0's qk blocks + early weight slices load first
    (first scores matmul at ~11us); v and the remaining qk pairs are emitted
    between attention heads as PE filler so the tensor engine stays HAM-warm
    (2.4 GHz) while the scalar engine paces the exp stream.
"""

import sys

sys.path.insert(0, "/opt/trn_rl_repo")

import numpy as np
import ml_dtypes

import concourse.bass as bass
from concourse import bacc
import concourse.mybir as mybir
import concourse.tile as tile
from concourse.bass_utils import run_bass_kernel_spmd

B, N, C, H = 8, 1024, 768, 12
HEAD_DIM = C // H  # 64
SCALE = HEAD_DIM**-0.5
ALPHA = [1, 2, 3]
P = 128
NB = N // 512  # 2 blocks of 512 along token axis
CK = C // P  # 6 contraction chunks
MC = N // P  # 8 key-token chunks
BF16 = mybir.dt.bfloat16
F32 = mybir.dt.float32

_CACHE = {}


def _build_nc():
    nc = bacc.Bacc(None, target_bir_lowering=False)
    xT_d = nc.declare_dram_parameter("xT", [C, N], BF16, isOutput=False)
    wqkvT_d = nc.declare_dram_parameter("wqkvT", [C, 3 * C], BF16, isOutput=False)
    wprojT_d = nc.declare_dram_parameter("wprojT", [C + 1, C], BF16, isOutput=False)
    bias_d = nc.declare_dram_parameter("bias", [C], F32, isOutput=False)
    out_d = nc.declare_dram_parameter("out", [N, C], F32, isOutput=True)

    EXP = mybir.ActivationFunctionType.Exp

    with tile.TileContext(nc) as tc:
        with (
            tc.tile_pool(name="consts", bufs=1) as consts,
            tc.tile_pool(name="qkp", bufs=1) as qkp,
            tc.tile_pool(name="vp", bufs=1) as vp,
            tc.tile_pool(name="mhap", bufs=1) as mhap,
            tc.tile_pool(name="expp", bufs=6) as expp,
            tc.tile_pool(name="recp", bufs=3) as recp,
            tc.tile_pool(name="outp", bufs=3) as outp,
            tc.tile_pool(name="ps_s", bufs=2, space="PSUM") as ps_s,
            tc.tile_pool(name="ps_f", bufs=1, space="PSUM") as ps_f,
            tc.tile_pool(name="ps_acc", bufs=3, space="PSUM") as ps_acc,
        ):
            # ---- load constants / inputs ----
            xT = [consts.tile([P, N], BF16, name=f"xT{c}", tag=f"xT{c}") for c in range(CK)]
            wq = [consts.tile([P, 3 * C], BF16, name=f"wq{c}", tag=f"wq{c}") for c in range(CK)]
            wp = [consts.tile([P, C], BF16, name=f"wp{c}", tag=f"wp{c}") for c in range(CK)]
            # dedicated early copies of the j=0 / j=6 weight slices so head
            # pair 0 unlocks after ~2MB of DMA instead of ~5MB
            wq0 = [consts.tile([P, P], BF16, name=f"wq0_{c}", tag=f"wq0_{c}") for c in range(CK)]
            wq6 = [consts.tile([P, P], BF16, name=f"wq6_{c}", tag=f"wq6_{c}") for c in range(CK)]
            # ramp-critical loads: split xT 2-way and fan the issue across
            # four engines' DGEs so neither queue BW (~15 GB/s/queue) nor
            # serial descriptor-gen on one engine delays the first matmul
            for c in range(CK):
                nc.sync.dma_start(out=xT[c][0:64, :], in_=xT_d[c * P : c * P + 64, :])
                nc.scalar.dma_start(out=xT[c][64:P, :], in_=xT_d[c * P + 64 : (c + 1) * P, :])
                nc.gpsimd.dma_start(out=wq0[c], in_=wqkvT_d[c * P : (c + 1) * P, 0:P])
                nc.gpsimd.dma_start(out=wq6[c], in_=wqkvT_d[c * P : (c + 1) * P, C : C + P])
            for c in range(CK):
                nc.sync.dma_start(out=wq[c], in_=wqkvT_d[c * P : (c + 1) * P, :])
            for c in range(CK):
                nc.sync.dma_start(out=wp[c], in_=wprojT_d[c * P : (c + 1) * P, :])
            bias_b = consts.tile([P, C], F32, tag="bias_b")
            _bap = bias_d[:]
            nc.sync.dma_start(
                out=bias_b,
                in_=bass.AP(tensor=_bap.tensor, offset=_bap.offset,
                            ap=[[0, P]] + list(_bap.ap)),
            )

            # HAM pre-warm: ~4.5us of dummy matmuls while the input DMAs are
            # still in flight, so the first real qk block runs at 2.4 GHz
            warm = consts.tile([P, 512], BF16, tag="warm")
            nc.vector.memset(warm, 0.0)
            for i in range(10):
                wpt = ps_f.tile([P, 512], F32, tag="mmf", name=f"warm{i}")
                nc.tensor.matmul(wpt, warm[:, 0:P], warm, start=True, stop=True)
            wsink = consts.tile([P, 512], BF16, tag="wsink")
            nc.vector.tensor_copy(out=wsink, in_=wpt)

            qkT = [qkp.tile([P, N], BF16, name=f"qkT{j}", tag=f"qkT{j}") for j in range(12)]
            vt = [vp.tile([P, H, 96], BF16, name=f"v{m}", tag=f"v{m}") for m in range(MC)]

            # qT/kT (j, n) block: one (128,1024) psum over both n-blocks;
            # consecutive matmuls share the lhsT (one LDWEIGHTS per (j,c))
            def emit_qk(j, wsrc=None, nbs=None):
                for nb in nbs if nbs is not None else range(NB):
                    pt = ps_f.tile([P, 512], F32, tag="mmf", name=f"pt_qk{j}_{nb}")
                    for c in range(CK):
                        w = wsrc[c] if wsrc is not None else wq[c][:, j * P : (j + 1) * P]
                        nc.tensor.matmul(
                            pt,
                            w,
                            xT[c][:, nb * 512 : (nb + 1) * 512],
                            start=(c == 0),
                            stop=(c == CK - 1),
                        )
                    nc.scalar.copy(out=qkT[j][:, nb * 512 : (nb + 1) * 512], in_=pt)

            # v natural (m, head, d) with ones column at d=64
            def emit_v(m):
                nc.vector.memset(vt[m][:, :, HEAD_DIM + 1 :], 0.0)
                nc.vector.memset(vt[m][:, :, HEAD_DIM], 1.0)
                for js, jn, h0 in ((2 * C, 512, 0), (2 * C + 512, 256, 8)):
                    pt = ps_f.tile([P, 512], F32, tag="mmf", name=f"pt_v{m}_{h0}")
                    for c in range(CK):
                        nc.tensor.matmul(
                            pt[:, :jn],
                            xT[c][:, m * P : (m + 1) * P],
                            wq[c][:, js : js + jn],
                            start=(c == 0),
                            stop=(c == CK - 1),
                        )
                    nc.scalar.copy(
                        out=vt[m][:, h0 : h0 + jn // HEAD_DIM, 0:HEAD_DIM],
                        in_=pt[:, :jn].rearrange("p (h d) -> p h d", d=HEAD_DIM),
                    )

            # head pair 0 unlocked first so ACT starts exp-ing early; v and
            # the remaining qk pairs are emitted INSIDE the attention sweep as
            # PE filler work, keeping the PE HAM-busy (2.4 GHz) while the
            # scalar engine paces the attention inner loop
            # head-pair-0 qk blocks run in the (idle) scores PSUM pool:
            # both nb halves per tile, two tiles in flight, no copy-drain
            # serialization on the ramp-critical path
            for j, wsrc in ((0, wq0), (6, wq6)):
                pt = ps_s.tile([P, N], F32, tag="mms", name=f"pt_qk{j}e")
                for c in range(CK):
                    for nb in range(NB):
                        nc.tensor.matmul(
                            pt[:, nb * 512 : (nb + 1) * 512],
                            wsrc[c],
                            xT[c][:, nb * 512 : (nb + 1) * 512],
                            start=(c == 0),
                            stop=(c == CK - 1),
                        )
                nc.vector.tensor_copy(out=qkT[j], in_=pt)

            # ---- attention per head (+ interleaved filler) ----
            mhaT = [mhap.tile([P, N], BF16, name=f"mhaT{c}", tag=f"mhaT{c}") for c in range(CK)]
            for h in range(H):
                r0 = (h % 2) * HEAD_DIM
                qtile = qkT[h // 2]
                ktile = qkT[6 + h // 2]
                # per-n-block accumulators (1 PSUM bank each, 3 bufs) so the
                # next head's attnV overlaps this head's normalize chain;
                # 96 partitions so [64:96] feeds the stream-transpose trick
                accs = [ps_acc.tile([96, 512], F32, tag="acc", name=f"acc{h}_{nb}") for nb in range(NB)]
                # software-pipeline skew: attnV(m) is emitted two iterations
                # behind scores(m), so the exp-wait that bacc attaches to
                # attnV's LDWEIGHTS has cleared long before the PE reaches it
                # (removes the ~100ns weight-load stall on first-of-pair MMs)
                LAG = 2

                def emit_attnv(m, ex):
                    for nb in range(NB):
                        nc.tensor.matmul(
                            accs[nb][0:96, :],
                            vt[m][:, h, :],
                            ex[:, nb * 512 : (nb + 1) * 512],
                            start=(m == 0),
                            stop=(m == MC - 1),
                        )

                exs = {}
                for m in range(MC):
                    spt = ps_s.tile([P, N], F32, tag="mms")
                    for nb in range(NB):
                        nc.tensor.matmul(
                            spt[:, nb * 512 : (nb + 1) * 512],
                            ktile[r0 : r0 + HEAD_DIM, m * P : (m + 1) * P],
                            qtile[r0 : r0 + HEAD_DIM, nb * 512 : (nb + 1) * 512],
                            start=True,
                            stop=True,
                        )
                    ex = expp.tile([P, N], BF16, tag="exp")
                    nc.scalar.activation(out=ex, in_=spt, func=EXP, scale=SCALE)
                    exs[m] = ex
                    if h == 0:
                        emit_v(m)
                    if m >= LAG:
                        emit_attnv(m - LAG, exs.pop(m - LAG))
                for m in range(MC - LAG, MC):
                    emit_attnv(m, exs.pop(m))
                # reciprocal of the sums row via the 32x32 stream-transpose
                # trick: DVE reciprocal cost scales with free-size-per-lane,
                # so spread the 512 sums across 32 partitions first
                for nb in range(NB):
                    acc = accs[nb]
                    tt = recp.tile([32, 512], F32, tag="tt")
                    nc.vector.transpose(out=tt, in_=acc[HEAD_DIM : HEAD_DIM + 32, :])
                    tv = tt.rearrange("p (b q) -> p b q", q=32)[:, :, 0]
                    nc.vector.reciprocal(out=tv, in_=tv)
                    rec = recp.tile([32, 512], F32, tag="rec")
                    nc.vector.transpose(out=rec, in_=tt)
                    recb = recp.tile([HEAD_DIM, 512], F32, tag="recb")
                    nc.gpsimd.partition_broadcast(recb, rec[0:1, :])
                    nc.vector.tensor_mul(
                        mhaT[h // 2][r0 : r0 + HEAD_DIM, nb * 512 : (nb + 1) * 512],
                        acc[0:HEAD_DIM, :],
                        recb,
                    )
                # feed the next qk pair as PE filler, one j-block per head
                # boundary, so filler coverage extends to head 9 instead of 8
                if h < 10:
                    if h % 2 == 0:
                        emit_qk(h // 2 + 1)
                    else:
                        emit_qk(6 + (h - 1) // 2 + 1)

            # ---- proj + bias (reuses the scores PSUM pool, now idle) ----
            for nch in range(MC):
                pt = ps_s.tile([P, N], F32, tag="mms")
                for c in range(CK):
                    for jb, jn in ((0, 512), (1, 256)):
                        nc.tensor.matmul(
                            pt[:, jb * 512 : jb * 512 + jn],
                            mhaT[c][:, nch * P : (nch + 1) * P],
                            wp[c][:, jb * 512 : jb * 512 + jn],
                            start=(c == 0),
                            stop=(c == CK - 1),
                        )
                ot = outp.tile([P, C], F32, tag="ot")
                nc.vector.tensor_add(ot, pt[:, :C], bias_b)
                for r in range(0, P, 64):
                    nc.sync.dma_start(
                        out=out_d[nch * P + r : nch * P + r + 64, :],
                        in_=ot[r : r + 64, :],
                    )

    if not nc.is_finalized():
        nc.finalize()
    return nc


def _get_nc():
    if "nc" not in _CACHE:
        _CACHE["nc"] = _build_nc()
    return _CACHE["nc"]


def _host_prep(x, gw, W_qkv, W_proj, b_proj):
    bf = ml_dtypes.bfloat16
    wqkvT = np.ascontiguousarray(W_qkv.T).astype(bf)

    wpA = np.empty((C + 1, C), np.float32)
    wpA[:C] = W_proj.T
    wpA[C] = b_proj

    s = np.full((C, C), gw[0], np.float32)
    s[: C // 2, : C // 2] += gw[1]
    s[: C // 3, : C // 3] += gw[2]
    wpB = np.empty((C + 1, C), np.float32)
    wpB[:C] = W_proj.T * s
    bs = np.full((C,), gw[0], np.float32)
    bs[: C // 2] += gw[1]
    bs[: C // 3] += gw[2]
    wpB[C] = b_proj * bs

    biasA = b_proj.astype(np.float32)
    biasB = (b_proj * bs).astype(np.float32)
    wpA = wpA.astype(bf)
    wpB = wpB.astype(bf)

    in_maps = []
    for i in range(B):
        in_maps.append(
            {
                "xT": np.ascontiguousarray(x[i].T).astype(bf),
                "wqkvT": wqkvT,
                "wprojT": wpA if i < B // 2 else wpB,
                "bias": biasA if i < B // 2 else biasB,
            }
        )
    return in_maps


def _run(in_maps, **kw):
    nc = _get_nc()
    try:
        return run_bass_kernel_spmd(nc, in_maps, core_ids=list(range(B)), **kw)
    except Exception:
        # device occasionally comes up unrecoverable after an abrupt prior
        # teardown; one retry reloads the NEFF cleanly
        import time as _time

        _time.sleep(5)
        return run_bass_kernel_spmd(nc, in_maps, core_ids=list(range(B)), **kw)


def kernel(x, gumbel_weights, W_qkv, W_proj, b_proj):
    x = np.asarray(x, np.float32)
    gw = np.asarray(gumbel_weights, np.float32)
    W_qkv = np.asarray(W_qkv, np.float32)
    W_proj = np.asarray(W_proj, np.float32)
    b_proj = np.asarray(b_proj, np.float32)

    in_maps = _host_prep(x, gw, W_qkv, W_proj, b_proj)
    res = _run(in_maps)
    out = np.stack([np.asarray(res.results[i]["out"]) for i in range(B)], axis=0)

    # latency accumulator, mimicking the reference's f32 arithmetic exactly
    latency = np.float32(0.0)
    for i, a in enumerate(ALPHA):
        flops = (4 * N * N * C // H * H // a) + (2 * N * C // a * C // a)
        latency = latency + np.float32(flops) * gw[i]

    return out, latency, gw



# revision 47
# speedup vs baseline: 1.0244x; 1.0244x over previous
"""Trainium2 Bass kernel for nn_Attention_16174846837077 (sparse_attention).

Math restructuring vs the reference:
  - The reference runs MHA 4 times (1 large + 3 gumbel-gated small branches with
    heads >= H//alpha zeroed).  A zeroed head has q=k=v=0 => its MHA output is 0,
    so branch i's MHA equals the full MHA with channels >= C//alpha_i zeroed.
  - Therefore small_sum = sum_i gw_i * ((mha * mask_i) @ W_proj.T + b_proj) * colmask_i
    collapses into ONE matmul with a block-rescaled weight matrix:
        Wtilde[j,c] = W_proj[j,c] * (gw0 + gw1*[j<384][c<384] + gw2*[j<256][c<256])
        btilde[j]   = b_proj[j]   * (gw0 + gw1*[j<384]        + gw2*[j<256])
  - Every batch then needs: QKV proj, full 12-head attention, one output proj.
    Data-parallel: batch b -> core b.  Cores 0-3 use (W_proj, b_proj), cores 4-7
    use (Wtilde, btilde).  Same SPMD graph, different weight data.  No collectives.

Device layout choices (all matmuls contract over the partition axis):
  - x is fed pre-transposed as xT (C, N) bf16.
  - q,k computed in transposed layout (d, n) so scoresT = kT.T @ qT directly.
  - v computed in natural layout (m, head, 96): 64 data cols + a ones column
    at 64 + zero padding, so attnV gives (96, n): rows 0-63 unnormalized out^T,
    row 64 = softmax denominators (exp-sums), rows 65-95 zeros (these make the
    32-row stream-transpose slice fully initialized).  Softmax skips
    max-subtraction (|score*scale| < ~4 for this data distribution).
  - normalization: the sums row is spread over 32 partitions with a DVE 32x32
    stream-transpose (reciprocal cost scales with free-size-per-lane), DVE
    reciprocal on the strided view, transpose back, gpsimd partition_broadcast
    to 64 rows, one elementwise multiply -> mhaT (c, n).
  - proj consumes mhaT directly as the stationary operand; the bias is a
    DMA-partition-broadcast (128, C) tile folded into the PSUM-drain add.
  - scheduling: head pair 0's qk blocks + early weight slices load first
    (first scores matmul at ~11us); v and the remaining qk pairs are emitted
    between attention heads as PE filler so the tensor engine stays HAM-warm
    (2.4 GHz) while the scalar engine paces the exp stream.
"""

import sys

sys.path.insert(0, "/opt/trn_rl_repo")

import numpy as np
import ml_dtypes

import concourse.bass as bass
from concourse import bacc
import concourse.mybir as mybir
import concourse.tile as tile
from concourse.bass_utils import run_bass_kernel_spmd

B, N, C, H = 8, 1024, 768, 12
HEAD_DIM = C // H  # 64
SCALE = HEAD_DIM**-0.5
ALPHA = [1, 2, 3]
P = 128
NB = N // 512  # 2 blocks of 512 along token axis
CK = C // P  # 6 contraction chunks
MC = N // P  # 8 key-token chunks
BF16 = mybir.dt.bfloat16
F32 = mybir.dt.float32

_CACHE = {}


def _build_nc():
    nc = bacc.Bacc(None, target_bir_lowering=False)
    xT_d = nc.declare_dram_parameter("xT", [C, N], BF16, isOutput=False)
    wqkvT_d = nc.declare_dram_parameter("wqkvT", [C, 3 * C], BF16, isOutput=False)
    wprojT_d = nc.declare_dram_parameter("wprojT", [C + 1, C], BF16, isOutput=False)
    bias_d = nc.declare_dram_parameter("bias", [C], F32, isOutput=False)
    out_d = nc.declare_dram_parameter("out", [N, C], F32, isOutput=True)

    EXP = mybir.ActivationFunctionType.Exp

    with tile.TileContext(nc) as tc:
        with (
            tc.tile_pool(name="consts", bufs=1) as consts,
            tc.tile_pool(name="qkp", bufs=1) as qkp,
            tc.tile_pool(name="vp", bufs=1) as vp,
            tc.tile_pool(name="mhap", bufs=1) as mhap,
            tc.tile_pool(name="expp", bufs=6) as expp,
            tc.tile_pool(name="recp", bufs=3) as recp,
            tc.tile_pool(name="outp", bufs=3) as outp,
            tc.tile_pool(name="ps_s", bufs=2, space="PSUM") as ps_s,
            tc.tile_pool(name="ps_f", bufs=1, space="PSUM") as ps_f,
            tc.tile_pool(name="ps_acc", bufs=3, space="PSUM") as ps_acc,
        ):
            # ---- load constants / inputs ----
            xT = [consts.tile([P, N], BF16, name=f"xT{c}", tag=f"xT{c}") for c in range(CK)]
            wq = [consts.tile([P, 3 * C], BF16, name=f"wq{c}", tag=f"wq{c}") for c in range(CK)]
            wp = [consts.tile([P, C], BF16, name=f"wp{c}", tag=f"wp{c}") for c in range(CK)]
            # dedicated early copies of the j=0 / j=6 weight slices so head
            # pair 0 unlocks after ~2MB of DMA instead of ~5MB
            wq0 = [consts.tile([P, P], BF16, name=f"wq0_{c}", tag=f"wq0_{c}") for c in range(CK)]
            wq6 = [consts.tile([P, P], BF16, name=f"wq6_{c}", tag=f"wq6_{c}") for c in range(CK)]
            # ramp-critical loads: split xT 2-way and fan the issue across
            # four engines' DGEs so neither queue BW (~15 GB/s/queue) nor
            # serial descriptor-gen on one engine delays the first matmul
            for c in range(CK):
                nc.sync.dma_start(out=xT[c][0:64, :], in_=xT_d[c * P : c * P + 64, :])
                nc.scalar.dma_start(out=xT[c][64:P, :], in_=xT_d[c * P + 64 : (c + 1) * P, :])
                nc.gpsimd.dma_start(out=wq0[c], in_=wqkvT_d[c * P : (c + 1) * P, 0:P])
                nc.gpsimd.dma_start(out=wq6[c], in_=wqkvT_d[c * P : (c + 1) * P, C : C + P])
            for c in range(CK):
                nc.sync.dma_start(out=wq[c], in_=wqkvT_d[c * P : (c + 1) * P, :])
            for c in range(CK):
                nc.sync.dma_start(out=wp[c], in_=wprojT_d[c * P : (c + 1) * P, :])
            bias_b = consts.tile([P, C], F32, tag="bias_b")
            _bap = bias_d[:]
            nc.sync.dma_start(
                out=bias_b,
                in_=bass.AP(tensor=_bap.tensor, offset=_bap.offset,
                            ap=[[0, P]] + list(_bap.ap)),
            )

            # HAM pre-warm: ~4.5us of dummy matmuls while the input DMAs are
            # still in flight, so the first real qk block runs at 2.4 GHz
            warm = consts.tile([P, 512], BF16, tag="warm")
            nc.vector.memset(warm, 0.0)
            for i in range(10):
                wpt = ps_f.tile([P, 512], F32, tag="mmf", name=f"warm{i}")
                nc.tensor.matmul(wpt, warm[:, 0:P], warm, start=True, stop=True)
            wsink = consts.tile([P, 512], BF16, tag="wsink")
            nc.vector.tensor_copy(out=wsink, in_=wpt)

            qkT = [qkp.tile([P, N], BF16, name=f"qkT{j}", tag=f"qkT{j}") for j in range(12)]
            vt = [vp.tile([P, H, 96], BF16, name=f"v{m}", tag=f"v{m}") for m in range(MC)]

            # qT/kT (j, n) block: one (128,1024) psum over both n-blocks;
            # consecutive matmuls share the lhsT (one LDWEIGHTS per (j,c))
            def emit_qk(j, wsrc=None, nbs=None):
                for nb in nbs if nbs is not None else range(NB):
                    pt = ps_f.tile([P, 512], F32, tag="mmf", name=f"pt_qk{j}_{nb}")
                    for c in range(CK):
                        w = wsrc[c] if wsrc is not None else wq[c][:, j * P : (j + 1) * P]
                        nc.tensor.matmul(
                            pt,
                            w,
                            xT[c][:, nb * 512 : (nb + 1) * 512],
                            start=(c == 0),
                            stop=(c == CK - 1),
                        )
                    nc.scalar.copy(out=qkT[j][:, nb * 512 : (nb + 1) * 512], in_=pt)

            # v natural (m, head, d) with ones column at d=64
            def emit_v(m):
                nc.vector.memset(vt[m][:, :, HEAD_DIM + 1 :], 0.0)
                nc.vector.memset(vt[m][:, :, HEAD_DIM], 1.0)
                for js, jn, h0 in ((2 * C, 512, 0), (2 * C + 512, 256, 8)):
                    pt = ps_f.tile([P, 512], F32, tag="mmf", name=f"pt_v{m}_{h0}")
                    for c in range(CK):
                        nc.tensor.matmul(
                            pt[:, :jn],
                            xT[c][:, m * P : (m + 1) * P],
                            wq[c][:, js : js + jn],
                            start=(c == 0),
                            stop=(c == CK - 1),
                        )
                    nc.scalar.copy(
                        out=vt[m][:, h0 : h0 + jn // HEAD_DIM, 0:HEAD_DIM],
                        in_=pt[:, :jn].rearrange("p (h d) -> p h d", d=HEAD_DIM),
                    )

            # head pair 0 unlocked first so ACT starts exp-ing early; v and
            # the remaining qk pairs are emitted INSIDE the attention sweep as
            # PE filler work, keeping the PE HAM-busy (2.4 GHz) while the
            # scalar engine paces the attention inner loop
            # head-pair-0 qk blocks run in the (idle) scores PSUM pool:
            # both nb halves per tile, two tiles in flight, no copy-drain
            # serialization on the ramp-critical path
            for j, wsrc in ((0, wq0), (6, wq6)):
                pt = ps_s.tile([P, N], F32, tag="mms", name=f"pt_qk{j}e")
                for c in range(CK):
                    for nb in range(NB):
                        nc.tensor.matmul(
                            pt[:, nb * 512 : (nb + 1) * 512],
                            wsrc[c],
                            xT[c][:, nb * 512 : (nb + 1) * 512],
                            start=(c == 0),
                            stop=(c == CK - 1),
                        )
                nc.vector.tensor_copy(out=qkT[j], in_=pt)

            # ---- attention per head (+ interleaved filler) ----
            mhaT = [mhap.tile([P, N], BF16, name=f"mhaT{c}", tag=f"mhaT{c}") for c in range(CK)]
            for h in range(H):
                r0 = (h % 2) * HEAD_DIM
                qtile = qkT[h // 2]
                ktile = qkT[6 + h // 2]
                # per-n-block accumulators (1 PSUM bank each, 3 bufs) so the
                # next head's attnV overlaps this head's normalize chain;
                # 96 partitions so [64:96] feeds the stream-transpose trick
                accs = [ps_acc.tile([96, 512], F32, tag="acc", name=f"acc{h}_{nb}") for nb in range(NB)]
                for m in range(MC):
                    spt = ps_s.tile([P, N], F32, tag="mms")
                    for nb in range(NB):
                        nc.tensor.matmul(
                            spt[:, nb * 512 : (nb + 1) * 512],
                            ktile[r0 : r0 + HEAD_DIM, m * P : (m + 1) * P],
                            qtile[r0 : r0 + HEAD_DIM, nb * 512 : (nb + 1) * 512],
                            start=True,
                            stop=True,
                        )
                    ex = expp.tile([P, N], BF16, tag="exp")
                    nc.scalar.activation(out=ex, in_=spt, func=EXP, scale=SCALE)
                    if h == 0:
                        emit_v(m)
                    for nb in range(NB):
                        nc.tensor.matmul(
                            accs[nb][0:96, :],
                            vt[m][:, h, :],
                            ex[:, nb * 512 : (nb + 1) * 512],
                            start=(m == 0),
                            stop=(m == MC - 1),
                        )
                # reciprocal of the sums row via the 32x32 stream-transpose
                # trick: DVE reciprocal cost scales with free-size-per-lane,
                # so spread the 512 sums across 32 partitions first
                for nb in range(NB):
                    acc = accs[nb]
                    tt = recp.tile([32, 512], F32, tag="tt")
                    nc.vector.transpose(out=tt, in_=acc[HEAD_DIM : HEAD_DIM + 32, :])
                    tv = tt.rearrange("p (b q) -> p b q", q=32)[:, :, 0]
                    nc.vector.reciprocal(out=tv, in_=tv)
                    rec = recp.tile([32, 512], F32, tag="rec")
                    nc.vector.transpose(out=rec, in_=tt)
                    recb = recp.tile([HEAD_DIM, 512], F32, tag="recb")
                    nc.gpsimd.partition_broadcast(recb, rec[0:1, :])
                    nc.vector.tensor_mul(
                        mhaT[h // 2][r0 : r0 + HEAD_DIM, nb * 512 : (nb + 1) * 512],
                        acc[0:HEAD_DIM, :],
                        recb,
                    )
                # feed the next qk pair as PE filler, one j-block per head
                # boundary, so filler coverage extends to head 9 instead of 8
                if h < 10:
                    if h % 2 == 0:
                        emit_qk(h // 2 + 1)
                    else:
                        emit_qk(6 + (h - 1) // 2 + 1)

            # ---- proj + bias (reuses the scores PSUM pool, now idle) ----
            for nch in range(MC):
                pt = ps_s.tile([P, N], F32, tag="mms")
                for c in range(CK):
                    for jb, jn in ((0, 512), (1, 256)):
                        nc.tensor.matmul(
                            pt[:, jb * 512 : jb * 512 + jn],
                            mhaT[c][:, nch * P : (nch + 1) * P],
                            wp[c][:, jb * 512 : jb * 512 + jn],
                            start=(c == 0),
                            stop=(c == CK - 1),
                        )
                ot = outp.tile([P, C], F32, tag="ot")
                nc.vector.tensor_add(ot, pt[:, :C], bias_b)
                for r in range(0, P, 64):
                    nc.sync.dma_start(
                        out=out_d[nch * P + r : nch * P + r + 64, :],
                        in_=ot[r : r + 64, :],
                    )

    if not nc.is_finalized():
        nc.finalize()
    return nc


def _get_nc():
    if "nc" not in _CACHE:
        _CACHE["nc"] = _build_nc()
    return _CACHE["nc"]


def _host_prep(x, gw, W_qkv, W_proj, b_proj):
    bf = ml_dtypes.bfloat16
    wqkvT = np.ascontiguousarray(W_qkv.T).astype(bf)

    wpA = np.empty((C + 1, C), np.float32)
    wpA[:C] = W_proj.T
    wpA[C] = b_proj

    s = np.full((C, C), gw[0], np.float32)
    s[: C // 2, : C // 2] += gw[1]
    s[: C // 3, : C // 3] += gw[2]
    wpB = np.empty((C + 1, C), np.float32)
    wpB[:C] = W_proj.T * s
    bs = np.full((C,), gw[0], np.float32)
    bs[: C // 2] += gw[1]
    bs[: C // 3] += gw[2]
    wpB[C] = b_proj * bs

    biasA = b_proj.astype(np.float32)
    biasB = (b_proj * bs).astype(np.float32)
    wpA = wpA.astype(bf)
    wpB = wpB.astype(bf)

    in_maps = []
    for i in range(B):
        in_maps.append(
            {
                "xT": np.ascontiguousarray(x[i].T).astype(bf),
                "wqkvT": wqkvT,
                "wprojT": wpA if i < B // 2 else wpB,
                "bias": biasA if i < B // 2 else biasB,
            }
        )
    return in_maps


def _run(in_maps, **kw):
    nc = _get_nc()
    try:
        return run_bass_kernel_spmd(nc, in_maps, core_ids=list(range(B)), **kw)
    except Exception:
        # device occasionally comes up unrecoverable after an abrupt prior
        # teardown; one retry reloads the NEFF cleanly
        import time as _time

        _time.sleep(5)
        return run_bass_kernel_spmd(nc, in_maps, core_ids=list(range(B)), **kw)


def kernel(x, gumbel_weights, W_qkv, W_proj, b_proj):
    x = np.asarray(x, np.float32)
    gw = np.asarray(gumbel_weights, np.float32)
    W_qkv = np.asarray(W_qkv, np.float32)
    W_proj = np.asarray(W_proj, np.float32)
    b_proj = np.asarray(b_proj, np.float32)

    in_maps = _host_prep(x, gw, W_qkv, W_proj, b_proj)
    res = _run(in_maps)
    out = np.stack([np.asarray(res.results[i]["out"]) for i in range(B)], axis=0)

    # latency accumulator, mimicking the reference's f32 arithmetic exactly
    latency = np.float32(0.0)
    for i, a in enumerate(ALPHA):
        flops = (4 * N * N * C // H * H // a) + (2 * N * C // a * C // a)
        latency = latency + np.float32(flops) * gw[i]

    return out, latency, gw

